# revision 1
# baseline (speedup 1.0000x reference)
"""DETM forward pass on 8 Trainium2 NeuronCores (Bass/Tile, SPMD).

Sharding: vocab V (and the V+K contraction of the theta MLP) is split 8
ways; the tiny sequential LSTM/eta chains are replicated on every core.
The beta softmax never materializes beta[times]: Z row-sums come free
from the exp pass (activation accum_out), and the nll becomes a dense
[B, T*K] @ [T*K, V/8] matmul against the kept exp(logits) tile using a
host-built one-hot mask.  The eta scan is a *linear* recurrence
(mu_t = A mu_{t-1} + u_t), evaluated in log-doubling form on the PE.
"""
import math
import os
import sys

for _p in ("/opt/trn_rl_repo", "/root/.axon_site/_ro/trn_rl_repo"):
    if _p not in sys.path and os.path.isdir(_p):
        sys.path.insert(0, _p)

import numpy as np
import ml_dtypes

import concourse.bass as bass
import concourse.bacc as bacc
import concourse.mybir as mybir
import concourse.tile as tile
from concourse.bass_utils import run_bass_kernel_spmd

F32 = mybir.dt.float32
BF16 = mybir.dt.bfloat16
FP8 = mybir.dt.float8e4
S2 = float(2.0 ** 20)  # pass-2 weight rescale so w=theta/Z fits fp8 range
AF = mybir.ActivationFunctionType
ALU = mybir.AluOpType

DELTA = 0.005
EPS = 1e-6
LOG_DELTA = math.log(DELTA)
K, T, V, TH, EH, E, L, B = 50, 40, 30000, 800, 200, 300, 3, 100
NC = 8
Vc = V // NC          # 3750 vocab columns per core
TK = T * K            # 2000
G4 = 4 * EH           # 800 raw gate width
GP = 1024             # padded gate width (4 blocks of 256)
NKC = 30              # contraction chunks for V-sharded matmuls (3840/128)
VPAD = NKC * 128      # 3840
NMT = 16              # row tiles of (t,k): 15x128 + 80
MROWS = [128] * 15 + [80]
ECH = [128, 128, 44]  # E=300 contraction chunks
LNS = 30000.0         # scale inside Ln to keep the LUT in a good range
# vocab tiles per core: 7x512 + 166
VTS = [512] * 7 + [Vc - 7 * 512]
MCH = [128] * 6 + [32]  # 800-row contraction chunks for mu/ls_theta


def _bf(x):
    return np.ascontiguousarray(x.astype(ml_dtypes.bfloat16))


def _f32(x):
    return np.ascontiguousarray(x.astype(np.float32))


def _pack(mat, cols, nch, dtype):
    """[R, cols] -> [128, nch*cols], chunk ch at cols [ch*cols:(ch+1)*cols]."""
    R = mat.shape[0]
    out = np.zeros((128, nch * cols), np.float32)
    for ch in range(nch):
        r0, r1 = ch * 128, min((ch + 1) * 128, R)
        if r0 >= R:
            break
        out[0 : r1 - r0, ch * cols : ch * cols + cols] = mat[r0:r1]
    return _bf(out) if dtype == "bf16" else _f32(out)


def prep_inputs(inputs):
    """Full inputs -> (list of 8 per-core input maps, host constants)."""
    f = {k: np.asarray(v) for k, v in inputs.items()}
    times = f["times"].astype(np.int64)

    # ---- replicated tensors --------------------------------------------
    alphas = np.transpose(f["mu_q_alpha"], (1, 0, 2)).reshape(TK, E)   # [(t,k), E]
    lsa = np.transpose(f["logsigma_q_alpha"], (1, 0, 2)).reshape(TK, E)
    aT = _pack(alphas.T, TK, 3, "bf16")                                # [128, 6000]

    # LSTM padded/reordered weights.  Gate blocks ordered [i, f, o, g] so
    # sigmoid covers psum cols 0:6 and tanh cols 6:8.
    blk_src = [0, 1, 3, 2]  # source block (i,f,g,o) for dest block (i,f,o,g)
    wiha = np.zeros((128, 3 * 2048), np.float32)
    whha = np.zeros((128, 3 * 2048), np.float32)
    binp = np.zeros((1, 4072), np.float32)  # all biases, applied via k=1 matmuls
    for l in range(L):
        Wp_i = np.zeros((GP, EH), np.float32)
        Wp_h = np.zeros((GP, EH), np.float32)
        bsum = f["lstm_bih"][l] + f["lstm_bhh"][l]
        for bd, bs in enumerate(blk_src):
            Wp_i[256 * bd : 256 * bd + EH] = f["lstm_Wih"][l][200 * bs : 200 * bs + 200]
            Wp_h[256 * bd : 256 * bd + EH] = f["lstm_Whh"][l][200 * bs : 200 * bs + 200]
            binp[0, l * 1024 + 256 * bd : l * 1024 + 256 * bd + EH] = bsum[
                200 * bs : 200 * bs + 200
            ]
        wT = Wp_i.T                                                    # [200, 1024]
        wiha[:, l * 2048 : l * 2048 + 1024] = wT[0:128]
        wiha[0:72, l * 2048 + 1024 : (l + 1) * 2048] = wT[128:200]
        whT = Wp_h.T                                                   # [200, 1024]
        whha[:, l * 2048 : l * 2048 + 1024] = whT[0:128]
        whha[0:72, l * 2048 + 1024 : (l + 1) * 2048] = whT[128:200]
    wiha = _bf(wiha)
    whha = _bf(whha)
    binp[0, 3072:3122] = f["b_mu_eta"]
    binp[0, 3122:3172] = f["b_ls_eta"]
    binp[0, 3172:3972] = f["b_theta"]
    binp[0, 3972:4022] = f["b_mu_theta"]
    binp[0, 4022:4072] = f["b_ls_theta"]
    binp = _bf(binp)
    onesrow = _bf(np.ones((1, 128), np.float32))

    # eta head: W_mu_eta = [Wo | We]
    woea = np.zeros((128, 200), np.float32)
    for j, (wname, bname) in enumerate(
        [("W_mu_eta", "b_mu_eta"), ("W_ls_eta", "b_ls_eta")]
    ):
        WoT = f[wname][:, 0:EH].T                                      # [200, 50]
        woea[:, 100 * j : 100 * j + 50] = WoT[0:128]
        woea[0:72, 100 * j + 50 : 100 * j + 100] = WoT[128:200]
    woea = _bf(woea)
    wem = np.zeros((50, 150), np.float32)
    wem[:, 0:50] = f["W_mu_eta"][:, EH:].T      # B0 = A^T
    wem[:, 50:100] = f["W_mu_eta"][:, EH:]      # P0 = A
    wem[:, 100:150] = f["W_ls_eta"][:, EH:].T   # We_ls^T
    wem = _f32(wem)

    divE = np.full((50, T), 1.0 / (DELTA + EPS), np.float32)
    divE[:, 0] = 1.0 / (1.0 + EPS)

    ohbtT = np.zeros((T, B), np.float32)        # one-hot times, transposed
    ohbtT[times, np.arange(B)] = 1.0
    ohb = np.zeros((128, T), np.float32)
    ohb[np.arange(B), times] = 1.0

    wthea = np.zeros((50, TH), np.float32)
    wthea[0:K] = f["W_theta"][:, V : V + K].T
    wthea = _bf(wthea)

    wmls = np.zeros((128, 700), np.float32)
    for j, wname in enumerate(["W_mu_theta", "W_ls_theta"]):
        wT = f[wname].T                                                # [800, 50]
        for ch in range(7):
            r0, r1 = ch * 128, min((ch + 1) * 128, 800)
            wmls[0 : r1 - r0, 350 * j + 50 * ch : 350 * j + 50 * ch + 50] = wT[r0:r1]
    wmls = _bf(wmls)

    i50 = _f32(np.eye(50))
    i128f = _f32(np.eye(128))
    onescol = _f32(np.ones((128, 1)))
    bmap = np.zeros((128, EH), np.float32)
    bmap[0:T] = np.broadcast_to(f["b_map"], (T, EH))
    bmap = _f32(bmap)

    # ---- per-core slices ----------------------------------------------
    weT_full = f["word_emb"].T                                         # [300, V]
    nb_T = f["normalized_bows"].T                                      # [V, B]
    wth_T = f["W_theta"][:, 0:V].T                                     # [V, 800]
    ri_T = f["rnn_inp"].T                                              # [V, 40]
    wm_T = f["W_map"].T                                                # [V, 200]

    lsaT = lsa.T                                                       # [300, 2000]
    muT = alphas.T                                                     # [300, 2000]
    JC = TK // NC                                                      # 250 cols/core

    in_maps = []
    for c in range(NC):
        v0, v1 = c * Vc, (c + 1) * Vc
        pad = np.zeros((VPAD - Vc, 1), np.float32)

        def padv(m):  # [Vc, X] -> [3840, X]
            return np.concatenate([m, np.zeros((VPAD - Vc, m.shape[1]), m.dtype)], 0)

        j0, j1 = c * JC, (c + 1) * JC
        muJ = muT[:, j0:j1]
        lsJ = lsaT[:, j0:j1]
        muJm = np.zeros((E, JC), np.float32)
        src_lo = j0 - K
        for jj in range(JC):
            s = src_lo + jj
            if s >= 0:
                muJm[:, jj] = muT[:, s]
        divA = np.zeros((E, JC), np.float32)
        for jj in range(JC):
            divA[:, jj] = (1.0 / (1.0 + EPS)) if (j0 + jj) < K else (
                1.0 / (DELTA + EPS)
            )

        bows = np.zeros((128, Vc), np.float32)
        bows[0:B] = f["bows"][:, v0:v1]

        m = dict(
            aT=aT,
            weT=_pack(weT_full[:, v0:v1], Vc, 3, "bf16"),
            nbT=_pack(padv(nb_T[v0:v1]), B, NKC, "bf16"),
            wthT=_pack(padv(wth_T[v0:v1]), TH, NKC, "bf16"),
            bows=_bf(bows),
            riT=_pack(padv(ri_T[v0:v1]), T, NKC, "bf16"),
            wmT=_pack(padv(wm_T[v0:v1]), EH, NKC, "bf16"),
            bmap=bmap,
            wiha=wiha,
            whha=whha,
            binp=binp,
            onesrow=onesrow,
            woea=woea,
            wem=wem,
            divE=divE,
            ohbtT=_f32(ohbtT),
            ohb=_f32(ohb),
            wthea=wthea,
            wmls=wmls,
            i50=i50,
            i128f=i128f,
            onescol=onescol,
            muJ=_pack(muJ, JC, 3, "f32"),
            muJm=_pack(muJm, JC, 3, "f32"),
            lsJ=_pack(lsJ, JC, 3, "f32"),
            divA=_pack(divA, JC, 3, "f32"),
        )
        in_maps.append(m)

    consts = dict(
        kl_alpha_c=0.5 * (K * E * (-1.0) + (T - 1) * K * E * (LOG_DELTA - 1.0)),
        kl_eta_c=0.5 * (K * (-1.0) + (T - 1) * K * (LOG_DELTA - 1.0)),
        kl_theta_c=0.5 * (B * K * (-1.0)),
        nll_lnS=math.log(LNS) * float(np.sum(f["bows"])),
    )
    return in_maps, consts


# =====================================================================
# device program
# =====================================================================

def build():
    nc = bacc.Bacc("TRN2", target_bir_lowering=False, debug=False, num_devices=NC)

    def din(name, shape, dt=BF16):
        return nc.dram_tensor(name, shape, dt, kind="ExternalInput").ap()

    aT = din("aT", [128, 3 * TK])
    weT = din("weT", [128, 3 * Vc])
    nbT = din("nbT", [128, NKC * B])
    wthT = din("wthT", [128, NKC * TH])
    bows = din("bows", [128, Vc], BF16)
    riT = din("riT", [128, NKC * T])
    wmT = din("wmT", [128, NKC * EH])
    bmap = din("bmap", [128, EH], F32)
    wiha = din("wiha", [128, 3 * 2048])
    whha = din("whha", [128, 3 * 2048])
    binp = din("binp", [1, 4072])
    onesrow = din("onesrow", [1, 128])
    woea = din("woea", [128, 200])
    wem = din("wem", [50, 150], F32)
    divE = din("divE", [50, T], F32)
    ohbtT = din("ohbtT", [T, B], F32)
    ohb = din("ohb", [128, T], F32)
    wthea = din("wthea", [50, TH])
    wmls = din("wmls", [128, 700])
    i50 = din("i50", [50, 50], F32)
    i128f = din("i128f", [128, 128], F32)
    onescol = din("onescol", [128, 1], F32)
    muJ = din("muJ", [128, 750], F32)
    muJm = din("muJm", [128, 750], F32)
    lsJ = din("lsJ", [128, 750], F32)
    divA = din("divA", [128, 750], F32)

    scal = nc.dram_tensor("scal", [1, 8], F32, kind="ExternalOutput").ap()

    JC3 = 750

    with tile.TileContext(nc) as tc:
        import contextlib

        ctx = contextlib.ExitStack()
        with ctx:
            pool = ctx.enter_context(tc.tile_pool(name="res", bufs=1))
            stream = ctx.enter_context(tc.tile_pool(name="stream", bufs=3))
            small = ctx.enter_context(tc.tile_pool(name="small", bufs=2))
            scratch = ctx.enter_context(tc.tile_pool(name="scratch", bufs=1))
            lstmw = ctx.enter_context(tc.tile_pool(name="lstmw", bufs=2))
            psA = ctx.enter_context(tc.tile_pool(name="psA", bufs=3, space="PSUM"))
            psB = ctx.enter_context(tc.tile_pool(name="psB", bufs=1, space="PSUM"))
            psS = ctx.enter_context(tc.tile_pool(name="psS", bufs=1, space="PSUM"))
            dram = ctx.enter_context(tc.tile_pool(name="dram", bufs=1, space="DRAM"))

            def load(pool_, ap_, dt=None, tag=None):
                if tag is None:
                    tag = "ld_" + ap_.tensor.name
                t = pool_.tile(list(ap_.shape), dt or ap_.dtype, tag=tag)
                nc.sync.dma_start(t[:], ap_)
                return t

            # ---- resident SBUF tensors --------------------------------
            aT_s = load(pool, aT)
            weT_s = load(pool, weT)
            E_s = pool.tile([128, NMT * Vc], FP8)     # exp(logits), kept resident
            i128_s = load(pool, i128f)
            i50_s = load(pool, i50)
            ones_s = load(pool, onescol)
            ohbtT_s = load(pool, ohbtT)
            ohb_s = load(pool, ohb)
            wem_s = load(pool, wem)
            divE_s = load(pool, divE)
            woea_s = load(pool, woea)
            wthea_s = load(pool, wthea)
            wmls_s = load(pool, wmls)
            bmap_s = load(pool, bmap)
            binp_s = load(pool, binp)
            ones1_s = load(pool, onesrow)

            # =========================================================
            # 1. mm0: out0 = rnn_inp @ W_map.T  (V-sharded) -> AllReduce
            # =========================================================
            ps0 = psB.tile([40, EH], F32, tag="acc1")
            for j in range(NKC):
                ri_j = load(stream, riT[:, j * T : (j + 1) * T], tag="ri_j")
                wm_j = load(stream, wmT[:, j * EH : (j + 1) * EH], tag="wm_j")
                nc.tensor.matmul(
                    ps0[:, :],
                    ri_j[:],
                    wm_j[:],
                    start=(j == 0),
                    stop=(j == NKC - 1),
                )
            ar1s = small.tile([40, EH], F32, tag="ar1s")
            nc.scalar.copy(ar1s[:], ps0[:])
            ar1i = dram.tile([40, EH], F32)
            ar1o = dram.tile([40, EH], F32)
            nc.sync.dma_start(ar1i[:], ar1s[:])
            nc.gpsimd.collective_compute(
                "AllReduce", ALU.add, replica_groups=[list(range(NC))],
                ins=[ar1i.opt()], outs=[ar1o.opt()],
            )
            out0_s = pool.tile([40, EH], F32)
            nc.sync.dma_start(out0_s[:], ar1o[:])
            nc.vector.tensor_add(out0_s[:], out0_s[:], bmap_s[0:40, :])

            # =========================================================
            # 2. LSTM (replicated): 3 layers, sequential steps
            # =========================================================
            Hcols = []
            for l in range(L):
                wih_l = load(lstmw, wiha[:, l * 2048 : (l + 1) * 2048], tag="wih")
                whh_l = load(lstmw, whha[:, l * 2048 : (l + 1) * 2048], tag="whh")
                # X_aug chunks (lhs of IPT matmul: [k,40] tiles)
                xa0 = small.tile([128, T], BF16, tag="xa0")
                xa1 = small.tile([128, T], BF16, tag="xa1")
                if l == 0:
                    pt0 = psS.tile([128, T], F32, tag="rot")
                    nc.tensor.transpose(pt0[:], out0_s[:, 0:128], i128_s[0:40, 0:40])
                    nc.vector.tensor_copy(xa0[:, :], pt0[:, :])
                    pt1 = psS.tile([128, T], F32, tag="rot")
                    nc.tensor.transpose(
                        pt1[0:72, :], out0_s[:, 128:200], i128_s[0:40, 0:40]
                    )
                    nc.vector.tensor_copy(xa1[0:72, :], pt1[0:72, :])
                else:
                    Hp = Hcols[l - 1]
                    nc.vector.tensor_copy(
                        xa0[:, :],
                        bass.AP(Hp[:].tensor, Hp[:].offset, [Hp[:].ap[0], [2, T]]),
                    )
                    nc.vector.tensor_copy(
                        xa1[0:72, :],
                        bass.AP(Hp[:].tensor, Hp[:].offset + 1, [[Hp[:].ap[0][0], 72], [2, T]]),
                    )

                # IPT[(pad-gate), t] = wih.T @ X + bias x 1  -> [128, 8] per t
                psI = psB.tile([128, 320], F32, tag="acc1")
                first = True
                for kc, (kr, xa) in enumerate([(128, xa0), (72, xa1)]):
                    for j in range(8):
                        nc.tensor.matmul(
                            psI[:, j * T : (j + 1) * T],
                            wih_l[0:kr, kc * 1024 + j * 128 : kc * 1024 + (j + 1) * 128],
                            xa[0:kr, :],
                            start=first,
                            stop=False,
                            skip_group_check=True,
                        )
                        first = False
                for j in range(8):
                    nc.tensor.matmul(
                        psI[:, j * T : (j + 1) * T],
                        binp_s[0:1, l * 1024 + j * 128 : l * 1024 + (j + 1) * 128],
                        ones1_s[0:1, 0:T],
                        start=False,
                        stop=(j == 7),
                        skip_group_check=True,
                    )
                IPT = pool.tile([128, 320], F32, tag=f"IPT{l}")
                nc.vector.tensor_copy(
                    bass.AP(IPT[:].tensor, IPT[:].offset, [IPT[:].ap[0], [8, T], [1, 8]]),
                    bass.AP(psI[:].tensor, psI[:].offset, [psI[:].ap[0], [1, T], [T, 8]]),
                )

                Hc = pool.tile([128, 2 * T], BF16, tag=f"H{l}")
                c_s = small.tile([128, 2], F32, tag="c")
                nc.vector.memset(c_s[:], 0.0)
                for t in range(T):
                    gact = small.tile([128, 8], F32, tag="gact")
                    if t == 0:
                        gsum = IPT
                        g0 = 0
                    else:
                        psG = psS.tile([128, 8], F32, tag="rot")
                        first = True
                        for kc, kr in ((0, 128), (1, 72)):
                            rhs = Hc[0:kr, 2 * (t - 1) + kc : 2 * (t - 1) + kc + 1]
                            for j in range(8):
                                nc.tensor.matmul(
                                    psG[:, j : j + 1],
                                    whh_l[0:kr, kc * 1024 + j * 128 : kc * 1024 + (j + 1) * 128],
                                    rhs,
                                    start=first,
                                    stop=(kc == 1 and j == 7),
                                    skip_group_check=True,
                                )
                                first = False
                        gsum = small.tile([128, 8], F32, tag="gsum")
                        nc.vector.tensor_add(
                            gsum[:], psG[:], IPT[:, t * 8 : (t + 1) * 8]
                        )
                        g0 = 0
                    nc.scalar.activation(
                        gact[:, 0:6], gsum[:, g0 : g0 + 6], AF.Sigmoid
                    )
                    nc.scalar.activation(
                        gact[:, 6:8], gsum[:, g0 + 6 : g0 + 8], AF.Tanh
                    )
                    tmp = small.tile([128, 2], F32, tag="tmp")
                    nc.vector.tensor_mul(tmp[:], gact[:, 0:2], gact[:, 6:8])
                    c2 = small.tile([128, 2], F32, tag="c2")
                    nc.vector.tensor_mul(c2[:], gact[:, 2:4], c_s[:])
                    nc.vector.tensor_add(c_s[:], c2[:], tmp[:])
                    tc_s = small.tile([128, 2], F32, tag="tc")
                    nc.scalar.activation(tc_s[:], c_s[:], AF.Tanh)
                    nc.vector.tensor_mul(Hc[:, 2 * t : 2 * t + 2], gact[:, 4:6], tc_s[:])
                Hcols.append(Hc)

            # =========================================================
            # 3. eta chain (closed form) + kl_eta
            # =========================================================
            H2 = Hcols[2]
            xa40 = bass.AP(H2[:].tensor, H2[:].offset, [H2[:].ap[0], [2, T]])
            xa41 = small.tile([128, T], BF16, tag="xa1")
            nc.vector.tensor_copy(
                xa41[0:72, :],
                bass.AP(H2[:].tensor, H2[:].offset + 1, [[H2[:].ap[0][0], 72], [2, T]]),
            )
            U_ps = psS.tile([50, T], F32, tag="rot")
            nc.tensor.matmul(U_ps[:], woea_s[:, 0:50], xa40, start=True, stop=False)
            nc.tensor.matmul(U_ps[:], woea_s[0:72, 50:100], xa41[0:72, :], start=False, stop=False)
            nc.tensor.matmul(
                U_ps[:], binp_s[0:1, 3072:3122], ones1_s[0:1, 0:T],
                start=False, stop=True,
            )
            LS_ps = psS.tile([50, T], F32, tag="LSps")
            nc.tensor.matmul(LS_ps[:], woea_s[:, 100:150], xa40, start=True, stop=False)
            nc.tensor.matmul(LS_ps[:], woea_s[0:72, 150:200], xa41[0:72, :], start=False, stop=False)
            nc.tensor.matmul(
                LS_ps[:], binp_s[0:1, 3122:3172], ones1_s[0:1, 0:T],
                start=False, stop=False,
            )

            Ecur = small.tile([50, T], F32, tag="Ecur")
            nc.scalar.copy(Ecur[:], U_ps[:])
            Bcur = small.tile([50, 50], F32, tag="Bcur")
            nc.vector.tensor_copy(Bcur[:], wem_s[:, 0:50])
            Pcur = small.tile([50, 50], F32, tag="Pcur")
            nc.vector.tensor_copy(Pcur[:], wem_s[:, 50:100])
            s = 1
            while s < T:
                En_ps = psS.tile([50, T], F32, tag="rot")
                nc.tensor.matmul(En_ps[:], i50_s[:], Ecur[:], start=True, stop=False)
                nc.tensor.matmul(
                    En_ps[:, s:T], Bcur[:], Ecur[:, 0 : T - s], start=False, stop=True,
                    skip_group_check=True,
                )
                Ecur = small.tile([50, T], F32, tag="Ecur")
                nc.scalar.copy(Ecur[:], En_ps[:])
                if 2 * s < T:
                    Pn_ps = psS.tile([50, 50], F32, tag="rot")
                    nc.tensor.matmul(Pn_ps[:], Bcur[:], Pcur[:], start=True, stop=True)
                    Pcur = small.tile([50, 50], F32, tag="Pcur")
                    nc.scalar.copy(Pcur[:], Pn_ps[:])
                    Bn_ps = psS.tile([50, 50], F32, tag="rot")
                    nc.tensor.transpose(Bn_ps[:], Pcur[:], i50_s[:])
                    Bcur = small.tile([50, 50], F32, tag="Bcur")
                    nc.vector.tensor_copy(Bcur[:], Bn_ps[:])
                s *= 2
            etasT = Ecur  # [50, 40] f32

            nc.tensor.matmul(
                LS_ps[:, 1:T], wem_s[:, 100:150], etasT[:, 0 : T - 1],
                start=False, stop=True, skip_group_check=True,
            )
            LS_s = small.tile([50, T], F32, tag="LS")
            nc.scalar.copy(LS_s[:], LS_ps[:])

            # kl_eta partial: sum((exp(ls)+d^2)*div - ls)
            dm = small.tile([50, T], F32, tag="dm")
            nc.vector.tensor_copy(dm[:, 0:1], etasT[:, 0:1])
            nc.vector.tensor_sub(dm[:, 1:T], etasT[:, 1:T], etasT[:, 0 : T - 1])
            xe = small.tile([50, T], F32, tag="xe")
            nc.scalar.activation(xe[:], LS_s[:], AF.Exp)
            d2 = small.tile([50, T], F32, tag="d2")
            nc.vector.tensor_mul(d2[:], dm[:], dm[:])
            nc.vector.tensor_add(d2[:], d2[:], xe[:])
            nc.vector.tensor_mul(d2[:], d2[:], divE_s[:])
            scrE = scratch.tile([50, T], F32, tag="junk")
            accE = pool.tile([50, 1], F32, tag="accE")
            nc.vector.scalar_tensor_tensor(
                scrE[:], d2[:], 1.0, LS_s[:], ALU.mult, ALU.subtract, accum_out=accE[:]
            )

            # etas_row [40, 50] and eta_td
            per_ps = psS.tile([40, 50], F32, tag="rot")
            nc.tensor.transpose(per_ps[:], etasT[:], i50_s[:])
            etr = small.tile([40, 50], F32, tag="etr")
            nc.scalar.copy(etr[:], per_ps[:])
            etd_ps = psS.tile([100, 50], F32, tag="rot")
            nc.tensor.matmul(etd_ps[:], ohbtT_s[:], etr[:], start=True, stop=True)
            etd = pool.tile([100, 50], F32, tag="etd_s")
            nc.scalar.copy(etd[:], etd_ps[:])
            etdT_ps = psS.tile([50, 100], F32, tag="rot")
            nc.tensor.transpose(etdT_ps[:], etd[:], i128_s[0:100, 0:100])
            etdT = pool.tile([64, 100], BF16, tag="etdT_s")
            nc.vector.tensor_copy(etdT[0:50, :], etdT_ps[:])

            # =========================================================
            # 4. pass 1: E = exp(alphas @ word_embT), Z row-sums
            # =========================================================
            Zacc = pool.tile([128, NMT * 8], F32, tag="Zacc")
            for m in range(NMT):
                mr = MROWS[m]
                off = 0
                for n, nw in enumerate(VTS):
                    psP = psA.tile([128, 512], F32, tag="mm")
                    for kc in range(3):
                        nc.tensor.matmul(
                            psP[0:mr, 0:nw],
                            aT_s[0 : ECH[kc], kc * TK + m * 128 : kc * TK + m * 128 + mr],
                            weT_s[0 : ECH[kc], kc * Vc + off : kc * Vc + off + nw],
                            start=(kc == 0),
                            stop=(kc == 2),
                        )
                    nc.scalar.activation(
                        E_s[0:mr, m * Vc + off : m * Vc + off + nw],
                        psP[0:mr, 0:nw],
                        AF.Exp,
                        accum_out=Zacc[0:mr, m * 8 + n : m * 8 + n + 1],
                    )
                    off += nw
            Zp = small.tile([128, NMT], F32, tag="Zp")
            nc.vector.tensor_reduce(
                Zp[:],
                bass.AP(Zacc[:].tensor, Zacc[:].offset, [Zacc[:].ap[0], [8, NMT], [1, 8]]),
                mybir.AxisListType.X,
                ALU.add,
            )

            # =========================================================
            # 5. theta pre-activation (V-sharded) + combined AllReduce
            # =========================================================
            psH = psB.tile([100, TH], F32, tag="th800")
            for j in range(NKC):
                nb_j = load(stream, nbT[:, j * B : (j + 1) * B], tag="nb_j")
                wch = load(stream, wthT[:, j * TH : (j + 1) * TH], tag="wch")
                for h0, h1 in ((0, 512), (512, 800)):
                    nc.tensor.matmul(
                        psH[:, h0:h1],
                        nb_j[:],
                        wch[:, h0:h1],
                        start=(j == 0),
                        stop=(j == NKC - 1),
                    )
            hpre = scratch.tile([100, TH], F32, tag="hpre")
            nc.scalar.copy(hpre[:], psH[:])

            ar2i = dram.tile([TK + B * TH], F32)
            ar2o = dram.tile([TK + B * TH], F32)
            nc.sync.dma_start(
                bass.AP(ar2i[:].tensor, ar2i[:].offset, [[1, 128], [128, 15]]),
                Zp[:, 0:15],
            )
            nc.sync.dma_start(
                bass.AP(ar2i[:].tensor, ar2i[:].offset + 1920, [[1, 80]]),
                Zp[0:80, 15:16],
            )
            nc.sync.dma_start(
                bass.AP(ar2i[:].tensor, ar2i[:].offset + TK, [[TH, 100], [1, TH]]),
                hpre[:],
            )
            nc.gpsimd.collective_compute(
                "AllReduce", ALU.add, replica_groups=[list(range(NC))],
                ins=[ar2i.opt()], outs=[ar2o.opt()],
            )
            Zmat = small.tile([40, 50], F32, tag="Zmat")
            nc.sync.dma_start(
                Zmat[:], bass.AP(ar2o[:].tensor, ar2o[:].offset, [[50, 40], [1, 50]])
            )
            hprs = scratch.tile([100, TH], F32, tag="hprs")
            nc.sync.dma_start(
                hprs[:],
                bass.AP(ar2o[:].tensor, ar2o[:].offset + TK, [[TH, 100], [1, TH]]),
            )

            # h = tanh(hpre + eta_td @ W2th.T + b_theta)
            psHE = psB.tile([100, TH], F32, tag="th800")
            for h0, h1 in ((0, 512), (512, 800)):
                nc.tensor.matmul(
                    psHE[:, h0:h1], etdT[0:50, :], wthea_s[0:50, h0:h1],
                    start=True, stop=False,
                )
                nc.tensor.matmul(
                    psHE[:, h0:h1], ones1_s[0:1, 0:100],
                    binp_s[0:1, 3172 + h0 : 3172 + h1],
                    start=False, stop=True,
                )
            hsum = scratch.tile([100, TH], F32, tag="hsum")
            nc.vector.tensor_add(hsum[:], hprs[:], psHE[:])
            h_s = pool.tile([100, TH], F32, tag="h_s")
            nc.scalar.activation(h_s[:], hsum[:], AF.Tanh)

            # hT (augmented with ones row in chunk 6)
            hT = pool.tile([128, 700], BF16, tag="hT")
            for j in range(7):
                w = 128 if j < 6 else 32
                psT = psS.tile([128, 100], F32, tag="rot")
                nc.tensor.transpose(
                    psT[0:w, :], h_s[:, j * 128 : j * 128 + w], i128_s[0:100, 0:100]
                )
                nc.vector.tensor_copy(hT[0:w, j * 100 : (j + 1) * 100], psT[0:w, :])

            psMT = psS.tile([100, 50], F32, tag="LSps")
            psLT = psS.tile([100, 50], F32, tag="rot")
            for j in range(7):
                kr = MCH[j]
                nc.tensor.matmul(
                    psMT[:], hT[0:kr, j * 100 : (j + 1) * 100],
                    wmls_s[0:kr, j * 50 : j * 50 + 50],
                    start=(j == 0), stop=False,
                )
            nc.tensor.matmul(
                psMT[:], ones1_s[0:1, 0:100], binp_s[0:1, 3972:4022],
                start=False, stop=True,
            )
            for j in range(7):
                kr = MCH[j]
                nc.tensor.matmul(
                    psLT[:], hT[0:kr, j * 100 : (j + 1) * 100],
                    wmls_s[0:kr, 350 + j * 50 : 350 + j * 50 + 50],
                    start=(j == 0), stop=False,
                )
            nc.tensor.matmul(
                psLT[:], ones1_s[0:1, 0:100], binp_s[0:1, 4022:4072],
                start=False, stop=True,
            )

            # theta = softmax(mu_t); kl_theta partial
            mx = small.tile([100, 1], F32, tag="mx")
            nc.vector.tensor_reduce(mx[:], psMT[:], mybir.AxisListType.X, ALU.max)
            nmx = small.tile([100, 1], F32, tag="nmx")
            nc.vector.tensor_scalar_mul(nmx[:], mx[:], -1.0)
            texp = small.tile([100, 50], F32, tag="texp")
            se = small.tile([100, 1], F32, tag="se")
            nc.scalar.activation(texp[:], psMT[:], AF.Exp, bias=nmx[:], accum_out=se[:])
            rse = small.tile([100, 1], F32, tag="rse")
            nc.vector.reciprocal(rse[:], se[:])
            theta = small.tile([100, 50], F32, tag="theta")
            nc.vector.tensor_scalar_mul(theta[:], texp[:], rse[:])

            lse = small.tile([100, 50], F32, tag="lse")
            nc.scalar.activation(lse[:], psLT[:], AF.Exp)
            dth = small.tile([100, 50], F32, tag="dth")
            nc.vector.tensor_sub(dth[:], psMT[:], etd[:])
            nc.vector.tensor_mul(dth[:], dth[:], dth[:])
            nc.vector.tensor_add(dth[:], dth[:], lse[:])
            scrT = scratch.tile([100, 50], F32, tag="junk")
            accT = pool.tile([100, 1], F32, tag="accT")
            nc.vector.scalar_tensor_tensor(
                scrT[:], dth[:], 1.0 / (1.0 + EPS), psLT[:], ALU.mult, ALU.subtract,
                accum_out=accT[:],
            )

            # w = theta / Z[times]; W2 = one-hot-expanded w
            zg_ps = psS.tile([100, 50], F32, tag="rot")
            nc.tensor.matmul(zg_ps[:], ohbtT_s[:], Zmat[:], start=True, stop=True)
            rzg = small.tile([100, 50], F32, tag="rzg")
            nc.vector.reciprocal(rzg[:], zg_ps[:])
            w_s = small.tile([100, 50], F32, tag="w_s")
            nc.vector.tensor_mul(w_s[:], theta[:], rzg[:])
            nc.vector.tensor_scalar_mul(w_s[:], w_s[:], S2)

            W2 = scratch.tile([100, TK], F32, tag="W2")
            nc.vector.tensor_tensor(
                bass.AP(W2[:].tensor, W2[:].offset, [W2[:].ap[0], [K, T], [1, K]]),
                bass.AP(w_s[:].tensor, w_s[:].offset, [w_s[:].ap[0], [0, T], [1, K]]),
                bass.AP(ohb_s[:].tensor, ohb_s[:].offset, [[ohb_s[:].ap[0][0], 100], [1, T], [0, K]]),
                ALU.mult,
            )
            w2T = pool.tile([128, NMT * 100], FP8, tag="w2T")
            for kc in range(NMT):
                kr = MROWS[kc]
                psW = psS.tile([128, 100], F32, tag="rot")
                nc.tensor.transpose(
                    psW[0:kr, :], W2[:, kc * 128 : kc * 128 + kr],
                    i128_s[0:100, 0:100],
                )
                nc.vector.tensor_copy(w2T[0:kr, kc * 100 : (kc + 1) * 100], psW[0:kr, :])

            # =========================================================
            # 6. pass 2: loglik = W2 @ E ; nll partial
            # =========================================================
            accN = pool.tile([100, 8], F32, tag="accN")
            biasEPS = pool.tile([128, 1], F32, tag="biasEPS")
            nc.gpsimd.memset(biasEPS[:], EPS * LNS)
            off = 0
            for n, nw in enumerate(VTS):
                psL = psA.tile([100, 512], F32, tag="mm")
                for kc in range(NMT):
                    kr = MROWS[kc]
                    nc.tensor.matmul(
                        psL[:, 0:nw],
                        w2T[0:kr, kc * 100 : (kc + 1) * 100],
                        E_s[0:kr, kc * Vc + off : kc * Vc + off + nw],
                        start=(kc == 0),
                        stop=(kc == NMT - 1),
                    )
                lg = stream.tile([100, 512], F32, tag="lg")
                nc.scalar.activation(
                    lg[:, 0:nw], psL[:, 0:nw], AF.Ln, bias=biasEPS[0:100, :],
                    scale=LNS / S2,
                )
                bw_n = load(stream, bows[0:100, off : off + nw], tag="bw_n")
                scrN = scratch.tile([100, 512], F32, tag="junk")
                nc.vector.scalar_tensor_tensor(
                    scrN[:, 0:nw], lg[:, 0:nw], 1.0, bw_n[:],
                    ALU.mult, ALU.mult, accum_out=accN[:, n : n + 1],
                )
                off += nw
            nacc = small.tile([100, 1], F32, tag="nacc")
            nc.vector.tensor_reduce(nacc[:], accN[:], mybir.AxisListType.X, ALU.add)

            # =========================================================
            # 7. kl_alpha partial (J-sharded)
            # =========================================================
            accA3 = small.tile([128, 3], F32, tag="accA3")
            for ci in range(3):
                c0, c1 = ci * 250, (ci + 1) * 250
                muJ_c = load(stream, muJ[:, c0:c1], tag="muJ_c")
                muJm_c = load(stream, muJm[:, c0:c1], tag="muJm_c")
                lsJ_c = load(stream, lsJ[:, c0:c1], tag="lsJ_c")
                divA_c = load(stream, divA[:, c0:c1], tag="divA_c")
                xeA = stream.tile([128, 250], F32, tag="xeA")
                nc.scalar.activation(xeA[:], lsJ_c[:], AF.Exp)
                dA = stream.tile([128, 250], F32, tag="dA")
                nc.vector.tensor_sub(dA[:], muJ_c[:], muJm_c[:])
                nc.vector.tensor_mul(dA[:], dA[:], dA[:])
                nc.vector.tensor_add(dA[:], dA[:], xeA[:])
                nc.vector.tensor_mul(dA[:], dA[:], divA_c[:])
                scrA = scratch.tile([128, 250], F32, tag="junk")
                nc.vector.scalar_tensor_tensor(
                    scrA[:], dA[:], 1.0, lsJ_c[:], ALU.mult, ALU.subtract,
                    accum_out=accA3[:, ci : ci + 1],
                )
            accA = pool.tile([128, 1], F32, tag="accA")
            nc.vector.tensor_reduce(accA[:], accA3[:], mybir.AxisListType.X, ALU.add)

            # =========================================================
            # 8. collect scalars -> scal[1, 8]
            # =========================================================
            psF = psB.tile([1, 8], F32, tag="acc1")
            parts = [
                (nacc, 100), (accA, 128), (accE, 50), (accT, 100),
            ]
            for i, (acc, p) in enumerate(parts):
                nc.tensor.matmul(
                    psF[0:1, i : i + 1], ones_s[0:p, :], acc[0:p, :],
                    start=(i == 0), stop=(i == len(parts) - 1),
                    skip_group_check=True,
                )
            scal_s = small.tile([1, 8], F32, tag="scal_s")
            nc.scalar.copy(scal_s[:], psF[:])
            nc.sync.dma_start(scal, scal_s[:])

    nc.compile()
    return nc


_NC_CACHE = None


def _get_nc():
    global _NC_CACHE
    if _NC_CACHE is None:
        _NC_CACHE = build()
    return _NC_CACHE


def kernel(**inputs):
    in_maps, consts = prep_inputs(inputs)
    nc = _get_nc()
    res = run_bass_kernel_spmd(nc, in_maps, list(range(NC))).results
    s = np.stack([r["scal"][0] for r in res])  # [8 cores, 8]
    nll = -(float(s[:, 0].sum()) - consts["nll_lnS"])
    kl_alpha = 0.5 * float(s[:, 1].sum()) + consts["kl_alpha_c"]
    kl_eta = 0.5 * float(s[0, 2]) + consts["kl_eta_c"]
    kl_theta = 0.5 * float(s[0, 3]) + consts["kl_theta_c"]
    nelbo = nll + kl_alpha + kl_eta + kl_theta
    return (
        np.float32(nelbo), np.float32(nll), np.float32(kl_alpha),
        np.float32(kl_eta), np.float32(kl_theta),
    )


if __name__ == "__main__":
    nc = build()
    print("built ok")
    from concourse.timeline_sim import TimelineSim
    tl = TimelineSim(nc)
    print("TimelineSim ns:", tl.simulate())

